# revision 36
# baseline (speedup 1.0000x reference)
"""Causal multi-head self-attention with RoPE on 8 TRN2 NeuronCores.

Sharding: batch (2) x head-groups (4 groups of 4 heads) -> 8 cores.
Each core computes q/k/v projections for its 4 heads from its batch slice,
runs causal attention, and a partial o_proj against the matching Wo column
block; the host sums the 4 partials per batch (the o_proj all-reduce).

Device-side structure (v2 — PE-stream optimized):
  * All activations live transposed (feature-major): xT [1024,2048],
    QT/KT [256,2048]; every matmul contraction sits on the partition axis.
  * Scores computed transposed ST[sk,sq] = K @ Q^T; softmax skips
    max-subtraction (scores bounded); exp(S^T) feeds the PV matmul.
  * The score pair for the two heads sharing a 128-row tile goes into ONE
    fused [128,1024] PSUM tile (2 banks); one fused ACT exp covers both
    halves for off-diagonal tiles (halves ACT instruction overhead).
  * V stored [seq, 128] per head-slot: cols 0..63 head dims, 64..127 ones
    (written once at startup via a strided memset) - PV yields OT rows 0..63
    and the softmax denominator on rows 64..127.
  * Normalization uses reciprocal_approx_fast (~5x the InstReciprocal rate;
    18 bits is plenty for a softmax denominator) and writes head-B's rows
    directly into ht[64:128] (cross-partition-base DVE write), removing the
    SBUF->SBUF staging DMA of v1.
  * PSUM->SBUF eviction moved off the DVE: V-projection and o_proj copies
    run on the idle GpSimd(Pool) engine.
  * The PE instruction stream is software-pipelined chunk-to-chunk: the
    next chunk's QKV projection chains and the previous chunk's o_proj are
    interleaved as filler between attention PV steps, so the tensor engine
    never drains (TRN2 DVFS: the PE only reaches 2.4 GHz after ~3us of
    continuous execution; any bubble drops it to 1.2 GHz).
  * RoPE applied in QT layout with head-dim pairs pre-permuted to
    [evens|odds] blocks; rotate-half is 32-partition block-swap DMAs.
  * Matmuls in fp16 (products exact in fp32 PSUM accumulation).
"""
import numpy as np
from collections import deque

import concourse.bass as bass
import concourse.mybir as mybir
import concourse.tile as tile
from concourse import bacc
from concourse.bass_utils import run_bass_kernel_spmd

F32 = mybir.dt.float32
F16 = mybir.dt.float16
AF = mybir.ActivationFunctionType
ALU = mybir.AluOpType

DT_MM = F16

BATCH, SEQ, DM = 2, 2048, 1024
NHEAD, DK = 16, 64
NCORES = 8
GROUPS = 4           # head groups (cores per batch)
HPC = 4              # heads per core
DH = HPC * DK        # 256 head dims per core
NK = DM // 128       # 8 contraction tiles over d_model
NJ = SEQ // 512      # 4 sq chunks
ROPE_THETA = 10000.0

TRACE = False
DEBUG_DUMP = False
LAST_RESULTS = None

_NC = None


def _build():
    nc = bacc.Bacc("TRN2", target_bir_lowering=False, debug=False)

    xt_d = nc.dram_tensor("xt", [DM, SEQ], DT_MM, kind="ExternalInput").ap()
    wq_d = nc.dram_tensor("wq", [DM, DH], DT_MM, kind="ExternalInput").ap()
    wk_d = nc.dram_tensor("wk", [DM, DH], DT_MM, kind="ExternalInput").ap()
    wv_d = nc.dram_tensor("wv", [DM, DH], DT_MM, kind="ExternalInput").ap()
    wo_d = nc.dram_tensor("wo", [DH, DM], DT_MM, kind="ExternalInput").ap()
    cos_d = nc.dram_tensor("cosf", [128, SEQ], F16, kind="ExternalInput").ap()
    sin_d = nc.dram_tensor("sinf", [128, SEQ], F16, kind="ExternalInput").ap()
    y_d = nc.dram_tensor("y", [SEQ, DM], F16, kind="ExternalOutput").ap()
    if DEBUG_DUMP:
        dbg_v = nc.dram_tensor("dbg_v", [128, 16 * (HPC * 128)], F16,
                               kind="ExternalOutput").ap()
        dbg_ht = nc.dram_tensor("dbg_ht", [128, 2 * SEQ], F16,
                                kind="ExternalOutput").ap()
        dbg_qt = nc.dram_tensor("dbg_qt", [128, 2 * SEQ], F16,
                                kind="ExternalOutput").ap()
        dbg_kt = nc.dram_tensor("dbg_kt", [128, 2 * SEQ], F16,
                                kind="ExternalOutput").ap()
        dbg_et = nc.dram_tensor("dbg_et", [128, 8 * 1024], F16,
                                kind="ExternalOutput").ap()
        dbg_ot = nc.dram_tensor("dbg_ot", [128, 2 * 512], F32,
                                kind="ExternalOutput").ap()

    with tile.TileContext(nc) as tc:
        with tc.tile_pool(name="persist", bufs=1) as pp, \
             tc.tile_pool(name="tabp", bufs=2) as tabp, \
             tc.tile_pool(name="ropep", bufs=2) as ropep, \
             tc.tile_pool(name="rcpp", bufs=4) as rcpp, \
             tc.tile_pool(name="etp", bufs=4) as etp, \
             tc.tile_pool(name="ysp", bufs=2) as ysp, \
             tc.tile_pool(name="ps_st", bufs=2, space="PSUM") as ps_st, \
             tc.tile_pool(name="ps_ot", bufs=2, space="PSUM") as ps_ot, \
             tc.tile_pool(name="ps_pj", bufs=2, space="PSUM") as ps_pj:

            # ---- resident tensors -------------------------------------
            qt = pp.tile([128, 2 * SEQ], DT_MM, tag="qt")
            kt = pp.tile([128, 2 * SEQ], DT_MM, tag="kt")
            v_sb = pp.tile([128, 16 * (HPC * 128)], DT_MM, tag="v")
            ht = pp.tile([128, 2 * SEQ], DT_MM, tag="ht")
            wo_sb = pp.tile([128, 2 * DM], DT_MM, tag="wo")
            xt = pp.tile([128, NK * SEQ], DT_MM, tag="xt")
            wq_sb = pp.tile([128, NK * DH], DT_MM, tag="wq")
            wk_sb = pp.tile([128, NK * DH], DT_MM, tag="wk")
            wv_sb = pp.tile([128, NK * DH], DT_MM, tag="wv")
            tri = pp.tile([128, 2 * 128], DT_MM, tag="tri")

            # ---- input DMAs -------------------------------------------
            # sync + gpsimd queues only (the scalar queue belongs to the
            # ACT engine, which is near-critical with the exp stream; the
            # Pool engine is nearly idle so it makes a good DMA driver).
            # Weights are partition-folded into single DMAs; xt is one
            # folded DMA per sq chunk so chunk c's columns land just in
            # time for its projections.
            xt_f = xt_d[:].rearrange("(k p) s -> p k s", p=128)
            xt_v = xt[:].rearrange("p (k s) -> p k s", s=SEQ)
            nc.sync.dma_start(
                out=wq_sb[:].rearrange("p (k d) -> p k d", d=DH),
                in_=wq_d[:].rearrange("(k p) d -> p k d", p=128))
            nc.gpsimd.dma_start(out=xt_v[:, :, 0:512], in_=xt_f[:, :, 0:512])
            nc.sync.dma_start(
                out=wk_sb[:].rearrange("p (k d) -> p k d", d=DH),
                in_=wk_d[:].rearrange("(k p) d -> p k d", p=128))
            cs_tiles, sn_tiles = {}, {}

            def load_tables(c):
                cs = tabp.tile([128, 512], F16, tag="cs")
                sn = tabp.tile([128, 512], F16, tag="sn")
                nc.sync.dma_start(out=cs[:], in_=cos_d[:, c * 512:(c + 1) * 512])
                nc.sync.dma_start(out=sn[:], in_=sin_d[:, c * 512:(c + 1) * 512])
                cs_tiles[c], sn_tiles[c] = cs, sn

            load_tables(0)
            nc.sync.dma_start(
                out=wv_sb[:].rearrange("p (k d) -> p k d", d=DH),
                in_=wv_d[:].rearrange("(k p) d -> p k d", p=128))
            for c in range(1, NJ):
                nc.gpsimd.dma_start(
                    out=xt_v[:, :, c * 512:(c + 1) * 512],
                    in_=xt_f[:, :, c * 512:(c + 1) * 512])
            nc.sync.dma_start(
                out=wo_sb[:].rearrange("p (k d) -> p k d", d=DM),
                in_=wo_d[:].rearrange("(k p) d -> p k d", p=128))

            # ones columns of every v head-slot, once. Layout [ones|data]:
            # the PV matmul then puts the softmax denominator on PSUM rows
            # 0:64 (base 0 — required by reciprocal_approx_fast, whose
            # custom-DVE ucode breaks at any non-zero base partition on HW)
            # and the head data on rows 64:128.
            v_ones = v_sb[:].rearrange("p (s d) -> p s d", d=128)[:, :, 0:64]
            nc.gpsimd.memset(v_ones, 1.0)

            # [128,128] lower-triangle 0/1 mask, duplicated side by side so
            # one op masks both heads' diagonal blocks. affine_select
            # mis-lowers on fp16 tiles on HW: build f32, cast to fp16.
            trif = pp.tile([128, 128], F32, tag="trif")
            nc.gpsimd.memset(trif[:], 1.0)
            nc.gpsimd.affine_select(out=trif[:], in_=trif[:],
                                    compare_op=ALU.is_ge, fill=0.0,
                                    base=0, pattern=[[1, 128]],
                                    channel_multiplier=-1)
            nc.gpsimd.tensor_copy(tri[:, 0:128], trif[:])
            nc.gpsimd.tensor_copy(tri[:, 128:256], trif[:])

            # ---- emission helpers -------------------------------------
            def qk_chain(c, dst, w_sb, m):
                def emit():
                    ps = ps_pj.tile([128, 512], F32, tag="pj")
                    for k in range(NK):
                        nc.tensor.matmul(
                            ps[:],
                            w_sb[:, k * DH + m * 128: k * DH + (m + 1) * 128],
                            xt[:, k * SEQ + c * 512: k * SEQ + (c + 1) * 512],
                            start=(k == 0), stop=(k == NK - 1))
                    nc.vector.tensor_copy(
                        dst[:, m * SEQ + c * 512: m * SEQ + (c + 1) * 512],
                        ps[:])
                return emit

            def v_chain(t):
                def emit():
                    ps = ps_pj.tile([128, 512], F32, tag="pj")
                    for k in range(NK):
                        nc.tensor.matmul(
                            ps[:, 0:DH],
                            xt[:, k * SEQ + t * 128: k * SEQ + t * 128 + 128],
                            wv_sb[:, k * DH:(k + 1) * DH],
                            start=(k == 0), stop=(k == NK - 1))
                    vv = v_sb[:, t * (HPC * 128):(t + 1) * (HPC * 128)
                              ].rearrange("p (h d) -> p h d", d=128)
                    nc.vector.tensor_copy(
                        vv[:, :, 64:128],
                        ps[:, 0:DH].rearrange("p (h d) -> p h d", d=64))
                return emit

            def rope(c):
                def emit():
                    cs, sn = cs_tiles.pop(c), sn_tiles.pop(c)
                    for si, src in enumerate((qt, kt)):
                        sview = src[:].rearrange("p (m S) -> p m S", m=2)[
                            :, :, c * 512:(c + 1) * 512]
                        sw = ropep.tile([128, 2 * 512], DT_MM, tag="sw")
                        swv = sw[:].rearrange("p (m S) -> p m S", m=2)
                        for blk in range(4):
                            sb_ = blk ^ 1
                            eng = nc.sync if (blk + si) % 2 == 0 else nc.gpsimd
                            eng.dma_start(
                                out=swv[blk * 32:(blk + 1) * 32, :, :],
                                in_=sview[sb_ * 32:(sb_ + 1) * 32, :, :])
                        for m in range(2):
                            seg = slice(m * SEQ + c * 512,
                                        m * SEQ + (c + 1) * 512)
                            t1 = ropep.tile([128, 512], F16, tag="t1")
                            nc.vector.tensor_mul(t1[:], src[:, seg], cs[:])
                            sw2 = ropep.tile([128, 512], F16, tag="sw2")
                            nc.vector.tensor_mul(sw2[:], swv[:, m, :], sn[:])
                            nc.vector.tensor_add(src[:, seg], t1[:], sw2[:])
                    if c + 2 < NJ and (c + 2) not in cs_tiles:
                        load_tables(c + 2)
                return emit

            def oproj_unit(j, t4, fine=False):
                # y[sq 128, dm 1024] partial for row-tile t4 of chunk j.
                # fine=True DMAs each 512-col half right after its copy
                # (shrinks the kernel tail on the last chunk).
                def emit():
                    ys = ysp.tile([128, 1024], F16, tag="ys")
                    rows = slice(j * 512 + t4 * 128, j * 512 + (t4 + 1) * 128)
                    for n in range(2):
                        ps = ps_pj.tile([128, 512], F32, tag="pj")
                        for kk in range(2):
                            nc.tensor.matmul(
                                ps[:],
                                ht[:, kk * SEQ + j * 512 + t4 * 128:
                                   kk * SEQ + j * 512 + (t4 + 1) * 128],
                                wo_sb[:, kk * DM + n * 512:
                                      kk * DM + (n + 1) * 512],
                                start=(kk == 0), stop=(kk == 1))
                        if n == 0:
                            nc.scalar.copy(ys[:, 0:512], ps[:])
                        else:
                            nc.vector.tensor_copy(ys[:, 512:1024], ps[:])
                        if fine:
                            eng = nc.gpsimd if (t4 + n) % 2 == 0 else nc.sync
                            eng.dma_start(
                                out=y_d[rows, n * 512:(n + 1) * 512],
                                in_=ys[:, n * 512:(n + 1) * 512])
                    if not fine:
                        eng = nc.gpsimd if t4 % 2 == 0 else nc.sync
                        eng.dma_start(out=y_d[rows, :], in_=ys[:])
                return emit

            def normalize(j, hp, otA, otB, split=1):
                # ot rows 0:64 = denominator (base 0), rows 64:128 = data.
                # split>1 emits the muls in column pieces so the last
                # chunk's o_proj can start on piece 0 while the rest run.
                jb = hp * SEQ + j * 512
                rcpA = rcpp.tile([64, 512], F32, tag="rcp")
                nc.vector.reciprocal_approx_fast(rcpA[:], otA[0:64, :])
                rcpB = rcpp.tile([64, 512], F32, tag="rcp")
                nc.vector.reciprocal_approx_fast(rcpB[:], otB[0:64, :])
                w = 512 // split
                for p in range(split):
                    lo, hi = p * w, (p + 1) * w
                    nc.vector.tensor_mul(ht[0:64, jb + lo:jb + hi],
                                         otA[64:128, lo:hi], rcpA[:, lo:hi])
                    nc.vector.tensor_mul(ht[64:128, jb + lo:jb + hi],
                                         otB[64:128, lo:hi], rcpB[:, lo:hi])

            # ---- main chunk loop --------------------------------------
            # chunk 0 (and chunk 1's QK+rope, since chunk 0's attention is
            # too short to hide them) emitted directly; everything else
            # arrives as filler inside the previous chunk's attention.
            for dst, w_sb in ((qt, wq_sb), (kt, wk_sb)):
                for m in range(2):
                    qk_chain(0, dst, w_sb, m)()
            if NJ > 1:
                load_tables(1)
            v_chain(0)()
            v_chain(1)()
            rope(0)()
            if NJ > 1:
                for dst, w_sb in ((qt, wq_sb), (kt, wk_sb)):
                    for m in range(2):
                        qk_chain(1, dst, w_sb, m)()
                rope(1)()

            for c in range(NJ):
                j = c
                nlive = 4 * (j + 1)
                # filler queues: QK of c+2 first (its copies gate the rope
                # filler), then V of c+1, then the previous chunk's o_proj;
                # rope(c+2) leads the hp1 queue.
                hp0_fill = deque()
                hp1_fill = deque()
                if c == 0:
                    hp0_fill.append(v_chain(2))
                    hp0_fill.append(v_chain(3))
                if c + 2 < NJ:
                    for dst, w_sb in ((qt, wq_sb), (kt, wk_sb)):
                        for m in range(2):
                            hp0_fill.append(qk_chain(c + 2, dst, w_sb, m))
                if c + 1 < NJ:
                    for t in range(4 * (c + 1), 4 * (c + 1) + 4):
                        hp0_fill.append(v_chain(t))
                if j > 0:
                    for t4 in range(2):
                        hp0_fill.append(oproj_unit(j - 1, t4))
                if c + 2 < NJ:
                    hp1_fill.append(rope(c + 2))
                if j > 0:
                    for t4 in range(2, 4):
                        hp1_fill.append(oproj_unit(j - 1, t4))

                for hp in range(2):
                    fill = hp0_fill if hp == 0 else hp1_fill
                    if hp == 1:
                        while hp0_fill:
                            hp1_fill.appendleft(hp0_fill.pop())
                    slots = nlive
                    otA = ps_ot.tile([128, 512], F32, tag="ot")
                    otB = ps_ot.tile([128, 512], F32, tag="ot")
                    jb = hp * SEQ + j * 512
                    ets = {}

                    def emit_st_exp(i, jb=jb, hp=hp, j=j, ets=ets):
                        r = i - 4 * j
                        c0 = 128 * r if r >= 0 else 0
                        ib = hp * SEQ + i * 128
                        st = ps_st.tile([128, 1024], F32, tag="st")
                        nc.tensor.matmul(st[:, c0:512],
                                         kt[0:64, ib:ib + 128],
                                         qt[0:64, jb + c0:jb + 512],
                                         start=True, stop=True)
                        nc.tensor.matmul(st[:, 512 + c0:1024],
                                         kt[64:128, ib:ib + 128],
                                         qt[64:128, jb + c0:jb + 512],
                                         start=True, stop=True)
                        et = etp.tile([128, 1024], DT_MM, tag="et")
                        if r >= 0:
                            stv = st[:].rearrange("p (g s) -> p g s", g=2)
                            etv = et[:].rearrange("p (g s) -> p g s", g=2)
                            nc.scalar.activation(etv[:, :, c0:512],
                                                 stv[:, :, c0:512],
                                                 AF.Exp, scale=0.125)
                            nc.vector.tensor_mul(
                                etv[:, :, c0:c0 + 128],
                                etv[:, :, c0:c0 + 128],
                                tri[:].rearrange("p (g d) -> p g d", g=2))
                        else:
                            nc.scalar.activation(et[:], st[:],
                                                 AF.Exp, scale=0.125)
                        ets[i] = (et, c0)
                        if DEBUG_DUMP and j == 1 and hp == 0:
                            nc.sync.dma_start(
                                out=dbg_et[:, i * 1024:(i + 1) * 1024],
                                in_=et[:])

                    def emit_pv(i, hp=hp, ets=ets, otA=otA, otB=otB,
                                nlive=nlive):
                        et, c0 = ets.pop(i)
                        vb = i * (HPC * 128) + 2 * hp * 128
                        nc.tensor.matmul(otA[:, c0:512],
                                         v_sb[:, vb:vb + 128],
                                         et[:, c0:512],
                                         start=(i == 0), stop=(i == nlive - 1))
                        nc.tensor.matmul(otB[:, c0:512],
                                         v_sb[:, vb + 128:vb + 256],
                                         et[:, 512 + c0:1024],
                                         start=(i == 0), stop=(i == nlive - 1))

                    n_fill0 = len(fill)
                    emitted = 0
                    for i in range(nlive):
                        emit_st_exp(i)
                        if i >= 1:
                            want = (n_fill0 * i + slots - 1) // slots
                            while fill and emitted < want:
                                fill.popleft()()
                                emitted += 1
                            emit_pv(i - 1)
                    while fill:
                        fill.popleft()()
                    emit_pv(nlive - 1)
                    if DEBUG_DUMP and j == 1 and hp == 0:
                        dbo = etp.tile([128, 1024], F32, tag="dbo",
                                       bufs=1)
                        nc.vector.tensor_copy(dbo[:, 0:512], otA[:])
                        nc.vector.tensor_copy(dbo[:, 512:1024], otB[:])
                        nc.sync.dma_start(out=dbg_ot[:], in_=dbo[:])
                    last = (j == NJ - 1 and hp == 1)
                    normalize(j, hp, otA, otB, split=4 if last else 1)

            for t4 in range(4):
                oproj_unit(NJ - 1, t4, fine=True)()

            if DEBUG_DUMP:
                nc.sync.dma_start(out=dbg_v[:], in_=v_sb[:])
                nc.sync.dma_start(out=dbg_ht[:], in_=ht[:])
                nc.sync.dma_start(out=dbg_qt[:], in_=qt[:])
                nc.sync.dma_start(out=dbg_kt[:], in_=kt[:])

    nc.compile()
    return nc


def _round_mm(a):
    return np.ascontiguousarray(a, dtype=np.float16)


def _prep_inputs(x, Wq, Wk, Wv, Wo, token_positions):
    x = np.asarray(x, dtype=np.float32)
    Wq = np.asarray(Wq, dtype=np.float32)
    Wk = np.asarray(Wk, dtype=np.float32)
    Wv = np.asarray(Wv, dtype=np.float32)
    Wo = np.asarray(Wo, dtype=np.float32)
    pos = np.asarray(token_positions).astype(np.float32)

    inv = 1.0 / (ROPE_THETA ** (np.arange(0, DK, 2, dtype=np.float32) / DK))
    freqs = pos[:, None] * inv[None, :]              # [SEQ, 32]
    cos_t, sin_t = np.cos(freqs).T, np.sin(freqs).T  # [32, SEQ]
    cosf = np.ascontiguousarray(np.tile(cos_t, (4, 1)), dtype=np.float16)
    sinf = np.tile(sin_t, (4, 1)).astype(np.float32)
    sinf[0:32] *= -1.0   # evens block gets -sin; odds +sin
    sinf[64:96] *= -1.0
    sinf = np.ascontiguousarray(sinf, dtype=np.float16)

    perm = np.concatenate([np.arange(0, 64, 2), np.arange(1, 64, 2)])
    in_maps = []
    for c in range(NCORES):
        b, g = divmod(c, GROUPS)
        rows = slice(g * DH, (g + 1) * DH)
        wq_s = Wq[rows, :].reshape(HPC, DK, DM)[:, perm, :].reshape(DH, DM)
        wk_s = Wk[rows, :].reshape(HPC, DK, DM)[:, perm, :].reshape(DH, DM)
        in_maps.append({
            "xt": _round_mm(x[b].T),
            "wq": _round_mm(wq_s.T),
            "wk": _round_mm(wk_s.T),
            "wv": _round_mm(Wv[rows, :].T),
            "wo": _round_mm(Wo[:, rows].T),
            "cosf": cosf,
            "sinf": sinf,
        })
    return in_maps


def kernel(x, Wq, Wk, Wv, Wo, token_positions):
    global _NC, LAST_RESULTS
    if _NC is None:
        _NC = _build()
    in_maps = _prep_inputs(x, Wq, Wk, Wv, Wo, token_positions)
    res = run_bass_kernel_spmd(_NC, in_maps, list(range(NCORES)), trace=TRACE)
    LAST_RESULTS = res
    y = np.empty((BATCH, SEQ, DM), dtype=np.float32)
    for b in range(BATCH):
        acc = res.results[4 * b]["y"].astype(np.float32).copy()
        for g in range(1, GROUPS):
            acc += res.results[4 * b + g]["y"]
        y[b] = acc
    return y


# revision 37
# speedup vs baseline: 1.1804x; 1.1804x over previous
"""Causal multi-head self-attention with RoPE on 8 TRN2 NeuronCores.

Sharding: batch (2) x head-groups (4 groups of 4 heads) -> 8 cores.
Each core computes q/k/v projections for its 4 heads from its batch slice,
runs causal attention, and a partial o_proj against the matching Wo column
block; the host sums the 4 partials per batch (the o_proj all-reduce).

Device-side structure (v2 — PE-stream optimized):
  * All activations live transposed (feature-major): xT [1024,2048],
    QT/KT [256,2048]; every matmul contraction sits on the partition axis.
  * Scores computed transposed ST[sk,sq] = K @ Q^T; softmax skips
    max-subtraction (scores bounded); exp(S^T) feeds the PV matmul.
  * The score pair for the two heads sharing a 128-row tile goes into ONE
    fused [128,1024] PSUM tile (2 banks); one fused ACT exp covers both
    halves for off-diagonal tiles (halves ACT instruction overhead).
  * V stored [seq, 128] per head-slot: cols 0..63 head dims, 64..127 ones
    (written once at startup via a strided memset) - PV yields OT rows 0..63
    and the softmax denominator on rows 64..127.
  * Normalization uses reciprocal_approx_fast (~5x the InstReciprocal rate;
    18 bits is plenty for a softmax denominator) and writes head-B's rows
    directly into ht[64:128] (cross-partition-base DVE write), removing the
    SBUF->SBUF staging DMA of v1.
  * PSUM->SBUF eviction moved off the DVE: V-projection and o_proj copies
    run on the idle GpSimd(Pool) engine.
  * The PE instruction stream is software-pipelined chunk-to-chunk: the
    next chunk's QKV projection chains and the previous chunk's o_proj are
    interleaved as filler between attention PV steps, so the tensor engine
    never drains (TRN2 DVFS: the PE only reaches 2.4 GHz after ~3us of
    continuous execution; any bubble drops it to 1.2 GHz).
  * RoPE applied in QT layout with head-dim pairs pre-permuted to
    [evens|odds] blocks; rotate-half is 32-partition block-swap DMAs.
  * Matmuls in fp16 (products exact in fp32 PSUM accumulation).
"""
import numpy as np
from collections import deque

import concourse.bass as bass
import concourse.mybir as mybir
import concourse.tile as tile
from concourse import bacc
from concourse.bass_utils import run_bass_kernel_spmd

F32 = mybir.dt.float32
F16 = mybir.dt.float16
AF = mybir.ActivationFunctionType
ALU = mybir.AluOpType

DT_MM = F16

BATCH, SEQ, DM = 2, 2048, 1024
NHEAD, DK = 16, 64
NCORES = 8
GROUPS = 4           # head groups (cores per batch)
HPC = 4              # heads per core
DH = HPC * DK        # 256 head dims per core
NK = DM // 128       # 8 contraction tiles over d_model
NJ = SEQ // 512      # 4 sq chunks
ROPE_THETA = 10000.0

TRACE = False
DEBUG_DUMP = False
LAST_RESULTS = None

_NC = None


def _build():
    nc = bacc.Bacc("TRN2", target_bir_lowering=False, debug=False)

    xt_d = nc.dram_tensor("xt", [DM, SEQ], DT_MM, kind="ExternalInput").ap()
    wq_d = nc.dram_tensor("wq", [DM, DH], DT_MM, kind="ExternalInput").ap()
    wk_d = nc.dram_tensor("wk", [DM, DH], DT_MM, kind="ExternalInput").ap()
    wv_d = nc.dram_tensor("wv", [DM, DH], DT_MM, kind="ExternalInput").ap()
    wo_d = nc.dram_tensor("wo", [DH, DM], DT_MM, kind="ExternalInput").ap()
    cos_d = nc.dram_tensor("cosf", [128, SEQ], F16, kind="ExternalInput").ap()
    sin_d = nc.dram_tensor("sinf", [128, SEQ], F16, kind="ExternalInput").ap()
    y_d = nc.dram_tensor("y", [SEQ, DM], F16, kind="ExternalOutput").ap()
    if DEBUG_DUMP:
        dbg_v = nc.dram_tensor("dbg_v", [128, 16 * (HPC * 128)], F16,
                               kind="ExternalOutput").ap()
        dbg_ht = nc.dram_tensor("dbg_ht", [128, 2 * SEQ], F16,
                                kind="ExternalOutput").ap()
        dbg_qt = nc.dram_tensor("dbg_qt", [128, 2 * SEQ], F16,
                                kind="ExternalOutput").ap()
        dbg_kt = nc.dram_tensor("dbg_kt", [128, 2 * SEQ], F16,
                                kind="ExternalOutput").ap()
        dbg_et = nc.dram_tensor("dbg_et", [128, 8 * 1024], F16,
                                kind="ExternalOutput").ap()
        dbg_ot = nc.dram_tensor("dbg_ot", [128, 2 * 512], F32,
                                kind="ExternalOutput").ap()

    with tile.TileContext(nc) as tc:
        with tc.tile_pool(name="persist", bufs=1) as pp, \
             tc.tile_pool(name="tabp", bufs=2) as tabp, \
             tc.tile_pool(name="ropep", bufs=2) as ropep, \
             tc.tile_pool(name="rcpp", bufs=4) as rcpp, \
             tc.tile_pool(name="etp", bufs=4) as etp, \
             tc.tile_pool(name="ysp", bufs=2) as ysp, \
             tc.tile_pool(name="ps_st", bufs=2, space="PSUM") as ps_st, \
             tc.tile_pool(name="ps_ot", bufs=2, space="PSUM") as ps_ot, \
             tc.tile_pool(name="ps_pj", bufs=2, space="PSUM") as ps_pj:

            # ---- resident tensors -------------------------------------
            qt = pp.tile([128, 2 * SEQ], DT_MM, tag="qt")
            kt = pp.tile([128, 2 * SEQ], DT_MM, tag="kt")
            v_sb = pp.tile([128, 16 * (HPC * 128)], DT_MM, tag="v")
            ht = pp.tile([128, 2 * SEQ], DT_MM, tag="ht")
            wo_sb = pp.tile([128, 2 * DM], DT_MM, tag="wo")
            xt = pp.tile([128, NK * SEQ], DT_MM, tag="xt")
            wq_sb = pp.tile([128, NK * DH], DT_MM, tag="wq")
            wk_sb = pp.tile([128, NK * DH], DT_MM, tag="wk")
            wv_sb = pp.tile([128, NK * DH], DT_MM, tag="wv")
            tri = pp.tile([128, 2 * 128], DT_MM, tag="tri")

            # ---- input DMAs -------------------------------------------
            # sync + gpsimd queues only (the scalar queue belongs to the
            # ACT engine, which is near-critical with the exp stream; the
            # Pool engine is nearly idle so it makes a good DMA driver).
            for k in range(NK):
                nc.sync.dma_start(out=wq_sb[:, k * DH:(k + 1) * DH],
                                  in_=wq_d[k * 128:(k + 1) * 128, :])
                nc.gpsimd.dma_start(out=xt[:, k * SEQ:k * SEQ + 512],
                                    in_=xt_d[k * 128:(k + 1) * 128, 0:512])
            for k in range(NK):
                nc.sync.dma_start(out=wk_sb[:, k * DH:(k + 1) * DH],
                                  in_=wk_d[k * 128:(k + 1) * 128, :])
            cs_tiles, sn_tiles = {}, {}

            def load_tables(c):
                cs = tabp.tile([128, 512], F16, tag="cs")
                sn = tabp.tile([128, 512], F16, tag="sn")
                nc.gpsimd.dma_start(out=cs[:], in_=cos_d[:, c * 512:(c + 1) * 512])
                nc.gpsimd.dma_start(out=sn[:], in_=sin_d[:, c * 512:(c + 1) * 512])
                cs_tiles[c], sn_tiles[c] = cs, sn

            load_tables(0)
            for k in range(NK):
                nc.gpsimd.dma_start(out=wv_sb[:, k * DH:(k + 1) * DH],
                                    in_=wv_d[k * 128:(k + 1) * 128, :])
            for c in range(1, NJ):
                for k in range(NK):
                    eng = nc.gpsimd if (k % 2 == 0) else nc.sync
                    eng.dma_start(
                        out=xt[:, k * SEQ + c * 512:k * SEQ + (c + 1) * 512],
                        in_=xt_d[k * 128:(k + 1) * 128, c * 512:(c + 1) * 512])
            for kk in range(2):
                nc.sync.dma_start(out=wo_sb[:, kk * DM:(kk + 1) * DM],
                                  in_=wo_d[kk * 128:(kk + 1) * 128, :])

            # ones columns of every v head-slot, once. Layout [ones|data]:
            # the PV matmul then puts the softmax denominator on PSUM rows
            # 0:64 (base 0 — required by reciprocal_approx_fast, whose
            # custom-DVE ucode breaks at any non-zero base partition on HW)
            # and the head data on rows 64:128.
            v_ones = v_sb[:].rearrange("p (s d) -> p s d", d=128)[:, :, 0:64]
            nc.gpsimd.memset(v_ones, 1.0)

            # [128,128] lower-triangle 0/1 mask, duplicated side by side so
            # one op masks both heads' diagonal blocks. affine_select
            # mis-lowers on fp16 tiles on HW: build f32, cast to fp16.
            trif = pp.tile([128, 128], F32, tag="trif")
            nc.gpsimd.memset(trif[:], 1.0)
            nc.gpsimd.affine_select(out=trif[:], in_=trif[:],
                                    compare_op=ALU.is_ge, fill=0.0,
                                    base=0, pattern=[[1, 128]],
                                    channel_multiplier=-1)
            nc.gpsimd.tensor_copy(tri[:, 0:128], trif[:])
            nc.gpsimd.tensor_copy(tri[:, 128:256], trif[:])

            # ---- emission helpers -------------------------------------
            def qk_chain(c, dst, w_sb, m):
                def emit():
                    ps = ps_pj.tile([128, 512], F32, tag="pj")
                    for k in range(NK):
                        nc.tensor.matmul(
                            ps[:],
                            w_sb[:, k * DH + m * 128: k * DH + (m + 1) * 128],
                            xt[:, k * SEQ + c * 512: k * SEQ + (c + 1) * 512],
                            start=(k == 0), stop=(k == NK - 1))
                    nc.vector.tensor_copy(
                        dst[:, m * SEQ + c * 512: m * SEQ + (c + 1) * 512],
                        ps[:])
                return emit

            def v_chain(t):
                def emit():
                    ps = ps_pj.tile([128, 512], F32, tag="pj")
                    for k in range(NK):
                        nc.tensor.matmul(
                            ps[:, 0:DH],
                            xt[:, k * SEQ + t * 128: k * SEQ + t * 128 + 128],
                            wv_sb[:, k * DH:(k + 1) * DH],
                            start=(k == 0), stop=(k == NK - 1))
                    vv = v_sb[:, t * (HPC * 128):(t + 1) * (HPC * 128)
                              ].rearrange("p (h d) -> p h d", d=128)
                    nc.vector.tensor_copy(
                        vv[:, :, 64:128],
                        ps[:, 0:DH].rearrange("p (h d) -> p h d", d=64))
                return emit

            def rope(c):
                def emit():
                    cs, sn = cs_tiles.pop(c), sn_tiles.pop(c)
                    for si, src in enumerate((qt, kt)):
                        sview = src[:].rearrange("p (m S) -> p m S", m=2)[
                            :, :, c * 512:(c + 1) * 512]
                        sw = ropep.tile([128, 2 * 512], DT_MM, tag="sw")
                        swv = sw[:].rearrange("p (m S) -> p m S", m=2)
                        for blk in range(4):
                            sb_ = blk ^ 1
                            eng = nc.sync if (blk + si) % 2 == 0 else nc.gpsimd
                            eng.dma_start(
                                out=swv[blk * 32:(blk + 1) * 32, :, :],
                                in_=sview[sb_ * 32:(sb_ + 1) * 32, :, :])
                        for m in range(2):
                            seg = slice(m * SEQ + c * 512,
                                        m * SEQ + (c + 1) * 512)
                            t1 = ropep.tile([128, 512], F16, tag="t1")
                            nc.vector.tensor_mul(t1[:], src[:, seg], cs[:])
                            sw2 = ropep.tile([128, 512], F16, tag="sw2")
                            nc.vector.tensor_mul(sw2[:], swv[:, m, :], sn[:])
                            nc.vector.tensor_add(src[:, seg], t1[:], sw2[:])
                    if c + 2 < NJ and (c + 2) not in cs_tiles:
                        load_tables(c + 2)
                return emit

            def oproj_unit(j, t4, fine=False):
                # y[sq 128, dm 1024] partial for row-tile t4 of chunk j.
                # fine=True DMAs each 512-col half right after its copy
                # (shrinks the kernel tail on the last chunk).
                def emit():
                    ys = ysp.tile([128, 1024], F16, tag="ys")
                    rows = slice(j * 512 + t4 * 128, j * 512 + (t4 + 1) * 128)
                    for n in range(2):
                        ps = ps_pj.tile([128, 512], F32, tag="pj")
                        for kk in range(2):
                            nc.tensor.matmul(
                                ps[:],
                                ht[:, kk * SEQ + j * 512 + t4 * 128:
                                   kk * SEQ + j * 512 + (t4 + 1) * 128],
                                wo_sb[:, kk * DM + n * 512:
                                      kk * DM + (n + 1) * 512],
                                start=(kk == 0), stop=(kk == 1))
                        if n == 0:
                            nc.scalar.copy(ys[:, 0:512], ps[:])
                        else:
                            nc.vector.tensor_copy(ys[:, 512:1024], ps[:])
                        if fine:
                            eng = nc.gpsimd if (t4 + n) % 2 == 0 else nc.sync
                            eng.dma_start(
                                out=y_d[rows, n * 512:(n + 1) * 512],
                                in_=ys[:, n * 512:(n + 1) * 512])
                    if not fine:
                        eng = nc.gpsimd if t4 % 2 == 0 else nc.sync
                        eng.dma_start(out=y_d[rows, :], in_=ys[:])
                return emit

            def normalize(j, hp, otA, otB, split=1):
                # ot rows 0:64 = denominator (base 0), rows 64:128 = data.
                # split>1 emits the muls in column pieces so the last
                # chunk's o_proj can start on piece 0 while the rest run.
                jb = hp * SEQ + j * 512
                rcpA = rcpp.tile([64, 512], F32, tag="rcp")
                nc.vector.reciprocal_approx_fast(rcpA[:], otA[0:64, :])
                rcpB = rcpp.tile([64, 512], F32, tag="rcp")
                nc.vector.reciprocal_approx_fast(rcpB[:], otB[0:64, :])
                w = 512 // split
                for p in range(split):
                    lo, hi = p * w, (p + 1) * w
                    nc.vector.tensor_mul(ht[0:64, jb + lo:jb + hi],
                                         otA[64:128, lo:hi], rcpA[:, lo:hi])
                    nc.vector.tensor_mul(ht[64:128, jb + lo:jb + hi],
                                         otB[64:128, lo:hi], rcpB[:, lo:hi])

            # ---- main chunk loop --------------------------------------
            # chunk 0 (and chunk 1's QK+rope, since chunk 0's attention is
            # too short to hide them) emitted directly; everything else
            # arrives as filler inside the previous chunk's attention.
            for dst, w_sb in ((qt, wq_sb), (kt, wk_sb)):
                for m in range(2):
                    qk_chain(0, dst, w_sb, m)()
            if NJ > 1:
                load_tables(1)
            v_chain(0)()
            v_chain(1)()
            rope(0)()
            if NJ > 1:
                for dst, w_sb in ((qt, wq_sb), (kt, wk_sb)):
                    for m in range(2):
                        qk_chain(1, dst, w_sb, m)()
                rope(1)()

            for c in range(NJ):
                j = c
                nlive = 4 * (j + 1)
                # filler queues: QK of c+2 first (its copies gate the rope
                # filler), then V of c+1, then the previous chunk's o_proj;
                # rope(c+2) leads the hp1 queue.
                hp0_fill = deque()
                hp1_fill = deque()
                if c == 0:
                    hp0_fill.append(v_chain(2))
                    hp0_fill.append(v_chain(3))
                if c + 2 < NJ:
                    for dst, w_sb in ((qt, wq_sb), (kt, wk_sb)):
                        for m in range(2):
                            hp0_fill.append(qk_chain(c + 2, dst, w_sb, m))
                if c + 1 < NJ:
                    for t in range(4 * (c + 1), 4 * (c + 1) + 4):
                        hp0_fill.append(v_chain(t))
                if j > 0:
                    for t4 in range(2):
                        hp0_fill.append(oproj_unit(j - 1, t4))
                if c + 2 < NJ:
                    hp1_fill.append(rope(c + 2))
                if j > 0:
                    for t4 in range(2, 4):
                        hp1_fill.append(oproj_unit(j - 1, t4))

                for hp in range(2):
                    fill = hp0_fill if hp == 0 else hp1_fill
                    if hp == 1:
                        while hp0_fill:
                            hp1_fill.appendleft(hp0_fill.pop())
                    slots = nlive
                    otA = ps_ot.tile([128, 512], F32, tag="ot")
                    otB = ps_ot.tile([128, 512], F32, tag="ot")
                    jb = hp * SEQ + j * 512
                    ets = {}

                    def emit_st_exp(i, jb=jb, hp=hp, j=j, ets=ets):
                        r = i - 4 * j
                        c0 = 128 * r if r >= 0 else 0
                        ib = hp * SEQ + i * 128
                        st = ps_st.tile([128, 1024], F32, tag="st")
                        nc.tensor.matmul(st[:, c0:512],
                                         kt[0:64, ib:ib + 128],
                                         qt[0:64, jb + c0:jb + 512],
                                         start=True, stop=True)
                        nc.tensor.matmul(st[:, 512 + c0:1024],
                                         kt[64:128, ib:ib + 128],
                                         qt[64:128, jb + c0:jb + 512],
                                         start=True, stop=True)
                        et = etp.tile([128, 1024], DT_MM, tag="et")
                        if r >= 0:
                            stv = st[:].rearrange("p (g s) -> p g s", g=2)
                            etv = et[:].rearrange("p (g s) -> p g s", g=2)
                            nc.scalar.activation(etv[:, :, c0:512],
                                                 stv[:, :, c0:512],
                                                 AF.Exp, scale=0.125)
                            nc.vector.tensor_mul(
                                etv[:, :, c0:c0 + 128],
                                etv[:, :, c0:c0 + 128],
                                tri[:].rearrange("p (g d) -> p g d", g=2))
                        else:
                            nc.scalar.activation(et[:], st[:],
                                                 AF.Exp, scale=0.125)
                        ets[i] = (et, c0)
                        if DEBUG_DUMP and j == 1 and hp == 0:
                            nc.sync.dma_start(
                                out=dbg_et[:, i * 1024:(i + 1) * 1024],
                                in_=et[:])

                    def emit_pv(i, hp=hp, ets=ets, otA=otA, otB=otB,
                                nlive=nlive):
                        et, c0 = ets.pop(i)
                        vb = i * (HPC * 128) + 2 * hp * 128
                        nc.tensor.matmul(otA[:, c0:512],
                                         v_sb[:, vb:vb + 128],
                                         et[:, c0:512],
                                         start=(i == 0), stop=(i == nlive - 1))
                        nc.tensor.matmul(otB[:, c0:512],
                                         v_sb[:, vb + 128:vb + 256],
                                         et[:, 512 + c0:1024],
                                         start=(i == 0), stop=(i == nlive - 1))

                    n_fill0 = len(fill)
                    emitted = 0
                    for i in range(nlive):
                        emit_st_exp(i)
                        if i >= 1:
                            want = (n_fill0 * i + slots - 1) // slots
                            while fill and emitted < want:
                                fill.popleft()()
                                emitted += 1
                            emit_pv(i - 1)
                    while fill:
                        fill.popleft()()
                    emit_pv(nlive - 1)
                    if DEBUG_DUMP and j == 1 and hp == 0:
                        dbo = etp.tile([128, 1024], F32, tag="dbo",
                                       bufs=1)
                        nc.vector.tensor_copy(dbo[:, 0:512], otA[:])
                        nc.vector.tensor_copy(dbo[:, 512:1024], otB[:])
                        nc.sync.dma_start(out=dbg_ot[:], in_=dbo[:])
                    last = (j == NJ - 1 and hp == 1)
                    normalize(j, hp, otA, otB, split=4 if last else 1)

            for t4 in range(4):
                oproj_unit(NJ - 1, t4, fine=True)()

            if DEBUG_DUMP:
                nc.sync.dma_start(out=dbg_v[:], in_=v_sb[:])
                nc.sync.dma_start(out=dbg_ht[:], in_=ht[:])
                nc.sync.dma_start(out=dbg_qt[:], in_=qt[:])
                nc.sync.dma_start(out=dbg_kt[:], in_=kt[:])

    nc.compile()
    return nc


def _round_mm(a):
    return np.ascontiguousarray(a, dtype=np.float16)


def _prep_inputs(x, Wq, Wk, Wv, Wo, token_positions):
    x = np.asarray(x, dtype=np.float32)
    Wq = np.asarray(Wq, dtype=np.float32)
    Wk = np.asarray(Wk, dtype=np.float32)
    Wv = np.asarray(Wv, dtype=np.float32)
    Wo = np.asarray(Wo, dtype=np.float32)
    pos = np.asarray(token_positions).astype(np.float32)

    inv = 1.0 / (ROPE_THETA ** (np.arange(0, DK, 2, dtype=np.float32) / DK))
    freqs = pos[:, None] * inv[None, :]              # [SEQ, 32]
    cos_t, sin_t = np.cos(freqs).T, np.sin(freqs).T  # [32, SEQ]
    cosf = np.ascontiguousarray(np.tile(cos_t, (4, 1)), dtype=np.float16)
    sinf = np.tile(sin_t, (4, 1)).astype(np.float32)
    sinf[0:32] *= -1.0   # evens block gets -sin; odds +sin
    sinf[64:96] *= -1.0
    sinf = np.ascontiguousarray(sinf, dtype=np.float16)

    perm = np.concatenate([np.arange(0, 64, 2), np.arange(1, 64, 2)])
    in_maps = []
    for c in range(NCORES):
        b, g = divmod(c, GROUPS)
        rows = slice(g * DH, (g + 1) * DH)
        wq_s = Wq[rows, :].reshape(HPC, DK, DM)[:, perm, :].reshape(DH, DM)
        wk_s = Wk[rows, :].reshape(HPC, DK, DM)[:, perm, :].reshape(DH, DM)
        in_maps.append({
            "xt": _round_mm(x[b].T),
            "wq": _round_mm(wq_s.T),
            "wk": _round_mm(wk_s.T),
            "wv": _round_mm(Wv[rows, :].T),
            "wo": _round_mm(Wo[:, rows].T),
            "cosf": cosf,
            "sinf": sinf,
        })
    return in_maps


def kernel(x, Wq, Wk, Wv, Wo, token_positions):
    global _NC, LAST_RESULTS
    if _NC is None:
        _NC = _build()
    in_maps = _prep_inputs(x, Wq, Wk, Wv, Wo, token_positions)
    res = run_bass_kernel_spmd(_NC, in_maps, list(range(NCORES)), trace=TRACE)
    LAST_RESULTS = res
    y = np.empty((BATCH, SEQ, DM), dtype=np.float32)
    for b in range(BATCH):
        acc = res.results[4 * b]["y"].astype(np.float32).copy()
        for g in range(1, GROUPS):
            acc += res.results[4 * b + g]["y"]
        y[b] = acc
    return y


# revision 41
# speedup vs baseline: 1.2025x; 1.0187x over previous
"""Causal multi-head self-attention with RoPE on 8 TRN2 NeuronCores.

Sharding: batch (2) x head-groups (4 groups of 4 heads) -> 8 cores.
Each core computes q/k/v projections for its 4 heads from its batch slice,
runs causal attention, and a partial o_proj against the matching Wo column
block; the host sums the 4 partials per batch (the o_proj all-reduce).

Device-side structure (v2 — PE-stream optimized):
  * All activations live transposed (feature-major): xT [1024,2048],
    QT/KT [256,2048]; every matmul contraction sits on the partition axis.
  * Scores computed transposed ST[sk,sq] = K @ Q^T; softmax skips
    max-subtraction (scores bounded); exp(S^T) feeds the PV matmul.
  * The score pair for the two heads sharing a 128-row tile goes into ONE
    fused [128,1024] PSUM tile (2 banks); one fused ACT exp covers both
    halves for off-diagonal tiles (halves ACT instruction overhead).
  * V stored [seq, 128] per head-slot: cols 0..63 head dims, 64..127 ones
    (written once at startup via a strided memset) - PV yields OT rows 0..63
    and the softmax denominator on rows 64..127.
  * Normalization uses reciprocal_approx_fast (~5x the InstReciprocal rate;
    18 bits is plenty for a softmax denominator) and writes head-B's rows
    directly into ht[64:128] (cross-partition-base DVE write), removing the
    SBUF->SBUF staging DMA of v1.
  * PSUM->SBUF eviction moved off the DVE: V-projection and o_proj copies
    run on the idle GpSimd(Pool) engine.
  * The PE instruction stream is software-pipelined chunk-to-chunk: the
    next chunk's QKV projection chains and the previous chunk's o_proj are
    interleaved as filler between attention PV steps, so the tensor engine
    never drains (TRN2 DVFS: the PE only reaches 2.4 GHz after ~3us of
    continuous execution; any bubble drops it to 1.2 GHz).
  * RoPE applied in QT layout with head-dim pairs pre-permuted to
    [evens|odds] blocks; rotate-half is 32-partition block-swap DMAs.
  * Matmuls in fp16 (products exact in fp32 PSUM accumulation).
"""
import numpy as np
from collections import deque

import concourse.bass as bass
import concourse.mybir as mybir
import concourse.tile as tile
from concourse import bacc
from concourse.bass_utils import run_bass_kernel_spmd

F32 = mybir.dt.float32
F16 = mybir.dt.float16
AF = mybir.ActivationFunctionType
ALU = mybir.AluOpType

DT_MM = F16

BATCH, SEQ, DM = 2, 2048, 1024
NHEAD, DK = 16, 64
NCORES = 8
GROUPS = 4           # head groups (cores per batch)
HPC = 4              # heads per core
DH = HPC * DK        # 256 head dims per core
NK = DM // 128       # 8 contraction tiles over d_model
NJ = SEQ // 512      # 4 sq chunks
ROPE_THETA = 10000.0

TRACE = False
DEBUG_DUMP = False
LAST_RESULTS = None

_NC = None


def _build():
    nc = bacc.Bacc("TRN2", target_bir_lowering=False, debug=False)

    xt_d = nc.dram_tensor("xt", [DM, SEQ], DT_MM, kind="ExternalInput").ap()
    wq_d = nc.dram_tensor("wq", [DM, DH], DT_MM, kind="ExternalInput").ap()
    wk_d = nc.dram_tensor("wk", [DM, DH], DT_MM, kind="ExternalInput").ap()
    wv_d = nc.dram_tensor("wv", [DM, DH], DT_MM, kind="ExternalInput").ap()
    wo_d = nc.dram_tensor("wo", [DH, DM], DT_MM, kind="ExternalInput").ap()
    cos_d = nc.dram_tensor("cosf", [128, SEQ], F16, kind="ExternalInput").ap()
    sin_d = nc.dram_tensor("sinf", [128, SEQ], F16, kind="ExternalInput").ap()
    y_d = nc.dram_tensor("y", [SEQ, DM], F16, kind="ExternalOutput").ap()
    if DEBUG_DUMP:
        dbg_v = nc.dram_tensor("dbg_v", [128, 16 * (HPC * 128)], F16,
                               kind="ExternalOutput").ap()
        dbg_ht = nc.dram_tensor("dbg_ht", [128, 2 * SEQ], F16,
                                kind="ExternalOutput").ap()
        dbg_qt = nc.dram_tensor("dbg_qt", [128, 2 * SEQ], F16,
                                kind="ExternalOutput").ap()
        dbg_kt = nc.dram_tensor("dbg_kt", [128, 2 * SEQ], F16,
                                kind="ExternalOutput").ap()
        dbg_et = nc.dram_tensor("dbg_et", [128, 8 * 1024], F16,
                                kind="ExternalOutput").ap()
        dbg_ot = nc.dram_tensor("dbg_ot", [128, 2 * 512], F32,
                                kind="ExternalOutput").ap()

    with tile.TileContext(nc) as tc:
        with tc.tile_pool(name="persist", bufs=1) as pp, \
             tc.tile_pool(name="tabp", bufs=2) as tabp, \
             tc.tile_pool(name="ropep", bufs=2) as ropep, \
             tc.tile_pool(name="rcpp", bufs=4) as rcpp, \
             tc.tile_pool(name="etp", bufs=4) as etp, \
             tc.tile_pool(name="ysp", bufs=2) as ysp, \
             tc.tile_pool(name="ps_st", bufs=2, space="PSUM") as ps_st, \
             tc.tile_pool(name="ps_ot", bufs=2, space="PSUM") as ps_ot, \
             tc.tile_pool(name="ps_pj", bufs=2, space="PSUM") as ps_pj:

            # ---- resident tensors -------------------------------------
            qt = pp.tile([128, 2 * SEQ], DT_MM, tag="qt")
            kt = pp.tile([128, 2 * SEQ], DT_MM, tag="kt")
            v_sb = pp.tile([128, 16 * (HPC * 128)], DT_MM, tag="v")
            ht = pp.tile([128, 2 * SEQ], DT_MM, tag="ht")
            wo_sb = pp.tile([128, 2 * DM], DT_MM, tag="wo")
            xt = pp.tile([128, NK * SEQ], DT_MM, tag="xt")
            wq_sb = pp.tile([128, NK * DH], DT_MM, tag="wq")
            wk_sb = pp.tile([128, NK * DH], DT_MM, tag="wk")
            wv_sb = pp.tile([128, NK * DH], DT_MM, tag="wv")
            tri = pp.tile([128, 2 * 128], DT_MM, tag="tri")

            # ---- input DMAs -------------------------------------------
            # sync + gpsimd queues only (the scalar queue belongs to the
            # ACT engine, which is near-critical with the exp stream; the
            # Pool engine is nearly idle so it makes a good DMA driver).
            # first chunk's wq/xt pairs split across both queues so chain k
            # has its inputs after ~k/2 transfer slots
            for k in range(NK):
                e1, e2 = (nc.sync, nc.gpsimd) if k % 2 == 0 else \
                         (nc.gpsimd, nc.sync)
                e1.dma_start(out=wq_sb[:, k * DH:(k + 1) * DH],
                             in_=wq_d[k * 128:(k + 1) * 128, :])
                e2.dma_start(out=xt[:, k * SEQ:k * SEQ + 512],
                             in_=xt_d[k * 128:(k + 1) * 128, 0:512])
            for k in range(NK):
                eng = nc.sync if k % 2 == 0 else nc.gpsimd
                eng.dma_start(out=wk_sb[:, k * DH:(k + 1) * DH],
                              in_=wk_d[k * 128:(k + 1) * 128, :])
            cs_tiles, sn_tiles = {}, {}

            def load_tables(c):
                cs = tabp.tile([128, 512], F16, tag="cs")
                sn = tabp.tile([128, 512], F16, tag="sn")
                nc.gpsimd.dma_start(out=cs[:], in_=cos_d[:, c * 512:(c + 1) * 512])
                nc.gpsimd.dma_start(out=sn[:], in_=sin_d[:, c * 512:(c + 1) * 512])
                cs_tiles[c], sn_tiles[c] = cs, sn

            load_tables(0)
            for k in range(NK):
                nc.gpsimd.dma_start(out=wv_sb[:, k * DH:(k + 1) * DH],
                                    in_=wv_d[k * 128:(k + 1) * 128, :])
            for c in range(1, NJ):
                for k in range(NK):
                    eng = nc.gpsimd if (k % 2 == 0) else nc.sync
                    eng.dma_start(
                        out=xt[:, k * SEQ + c * 512:k * SEQ + (c + 1) * 512],
                        in_=xt_d[k * 128:(k + 1) * 128, c * 512:(c + 1) * 512])
            for kk in range(2):
                nc.sync.dma_start(out=wo_sb[:, kk * DM:(kk + 1) * DM],
                                  in_=wo_d[kk * 128:(kk + 1) * 128, :])

            # ones columns of every v head-slot, once. Layout [ones|data]:
            # the PV matmul then puts the softmax denominator on PSUM rows
            # 0:64 (base 0 — required by reciprocal_approx_fast, whose
            # custom-DVE ucode breaks at any non-zero base partition on HW)
            # and the head data on rows 64:128.
            v_ones = v_sb[:].rearrange("p (s d) -> p s d", d=128)[:, :, 0:64]
            nc.gpsimd.memset(v_ones, 1.0)

            # [128,128] lower-triangle 0/1 mask, duplicated side by side so
            # one op masks both heads' diagonal blocks. affine_select
            # mis-lowers on fp16 tiles on HW: build f32, cast to fp16.
            trif = pp.tile([128, 128], F32, tag="trif")
            nc.gpsimd.memset(trif[:], 1.0)
            nc.gpsimd.affine_select(out=trif[:], in_=trif[:],
                                    compare_op=ALU.is_ge, fill=0.0,
                                    base=0, pattern=[[1, 128]],
                                    channel_multiplier=-1)
            nc.gpsimd.tensor_copy(tri[:, 0:128], trif[:])
            nc.gpsimd.tensor_copy(tri[:, 128:256], trif[:])

            # ---- emission helpers -------------------------------------
            def qk_chain(c, dst, w_sb, m):
                def emit():
                    ps = ps_pj.tile([128, 512], F32, tag="pj")
                    for k in range(NK):
                        nc.tensor.matmul(
                            ps[:],
                            w_sb[:, k * DH + m * 128: k * DH + (m + 1) * 128],
                            xt[:, k * SEQ + c * 512: k * SEQ + (c + 1) * 512],
                            start=(k == 0), stop=(k == NK - 1))
                    nc.vector.tensor_copy(
                        dst[:, m * SEQ + c * 512: m * SEQ + (c + 1) * 512],
                        ps[:])
                return emit

            def v_chain(t):
                def emit():
                    ps = ps_pj.tile([128, 512], F32, tag="pj")
                    for k in range(NK):
                        nc.tensor.matmul(
                            ps[:, 0:DH],
                            xt[:, k * SEQ + t * 128: k * SEQ + t * 128 + 128],
                            wv_sb[:, k * DH:(k + 1) * DH],
                            start=(k == 0), stop=(k == NK - 1))
                    vv = v_sb[:, t * (HPC * 128):(t + 1) * (HPC * 128)
                              ].rearrange("p (h d) -> p h d", d=128)
                    nc.vector.tensor_copy(
                        vv[:, :, 64:128],
                        ps[:, 0:DH].rearrange("p (h d) -> p h d", d=64))
                return emit

            def rope(c):
                def emit():
                    cs, sn = cs_tiles.pop(c), sn_tiles.pop(c)
                    for si, src in enumerate((qt, kt)):
                        sview = src[:].rearrange("p (m S) -> p m S", m=2)[
                            :, :, c * 512:(c + 1) * 512]
                        sw = ropep.tile([128, 2 * 512], DT_MM, tag="sw")
                        swv = sw[:].rearrange("p (m S) -> p m S", m=2)
                        for blk in range(4):
                            sb_ = blk ^ 1
                            eng = nc.sync if (blk + si) % 2 == 0 else nc.gpsimd
                            eng.dma_start(
                                out=swv[blk * 32:(blk + 1) * 32, :, :],
                                in_=sview[sb_ * 32:(sb_ + 1) * 32, :, :])
                        for m in range(2):
                            seg = slice(m * SEQ + c * 512,
                                        m * SEQ + (c + 1) * 512)
                            t1 = ropep.tile([128, 512], F16, tag="t1")
                            nc.vector.tensor_mul(t1[:], src[:, seg], cs[:])
                            sw2 = ropep.tile([128, 512], F16, tag="sw2")
                            nc.vector.tensor_mul(sw2[:], swv[:, m, :], sn[:])
                            nc.vector.tensor_add(src[:, seg], t1[:], sw2[:])
                    if c + 2 < NJ and (c + 2) not in cs_tiles:
                        load_tables(c + 2)
                return emit

            def oproj_unit(j, t4, fine=False):
                # y[sq 128, dm 1024] partial for row-tile t4 of chunk j.
                # fine=True DMAs each 512-col half right after its copy
                # (shrinks the kernel tail on the last chunk).
                def emit():
                    ys = ysp.tile([128, 1024], F16, tag="ys")
                    rows = slice(j * 512 + t4 * 128, j * 512 + (t4 + 1) * 128)
                    for n in range(2):
                        ps = ps_pj.tile([128, 512], F32, tag="pj")
                        for kk in range(2):
                            nc.tensor.matmul(
                                ps[:],
                                ht[:, kk * SEQ + j * 512 + t4 * 128:
                                   kk * SEQ + j * 512 + (t4 + 1) * 128],
                                wo_sb[:, kk * DM + n * 512:
                                      kk * DM + (n + 1) * 512],
                                start=(kk == 0), stop=(kk == 1))
                        if n == 0:
                            nc.scalar.copy(ys[:, 0:512], ps[:])
                        else:
                            nc.vector.tensor_copy(ys[:, 512:1024], ps[:])
                        if fine:
                            # keep the final DMAs off the gpsimd ring: its
                            # end-of-kernel DRAIN is slow when DMAs are
                            # still in flight there
                            eng = nc.gpsimd if t4 < 2 else nc.sync
                            eng.dma_start(
                                out=y_d[rows, n * 512:(n + 1) * 512],
                                in_=ys[:, n * 512:(n + 1) * 512])
                    if not fine:
                        eng = nc.gpsimd if t4 % 2 == 0 else nc.sync
                        eng.dma_start(out=y_d[rows, :], in_=ys[:])
                return emit

            def normalize(j, hp, otA, otB, split=1):
                # ot rows 0:64 = denominator (base 0), rows 64:128 = data.
                # split>1 emits the muls in column pieces so the last
                # chunk's o_proj can start on piece 0 while the rest run.
                jb = hp * SEQ + j * 512
                rcpA = rcpp.tile([64, 512], F32, tag="rcp")
                nc.vector.reciprocal_approx_fast(rcpA[:], otA[0:64, :])
                rcpB = rcpp.tile([64, 512], F32, tag="rcp")
                nc.vector.reciprocal_approx_fast(rcpB[:], otB[0:64, :])
                w = 512 // split
                for p in range(split):
                    lo, hi = p * w, (p + 1) * w
                    nc.vector.tensor_mul(ht[0:64, jb + lo:jb + hi],
                                         otA[64:128, lo:hi], rcpA[:, lo:hi])
                    nc.vector.tensor_mul(ht[64:128, jb + lo:jb + hi],
                                         otB[64:128, lo:hi], rcpB[:, lo:hi])

            # ---- main chunk loop --------------------------------------
            # chunk 0 (and chunk 1's QK+rope, since chunk 0's attention is
            # too short to hide them) emitted directly; everything else
            # arrives as filler inside the previous chunk's attention.
            for dst, w_sb in ((qt, wq_sb), (kt, wk_sb)):
                for m in range(2):
                    qk_chain(0, dst, w_sb, m)()
            if NJ > 1:
                load_tables(1)
            v_chain(0)()
            v_chain(1)()
            rope(0)()
            if NJ > 1:
                for dst, w_sb in ((qt, wq_sb), (kt, wk_sb)):
                    for m in range(2):
                        qk_chain(1, dst, w_sb, m)()

            for c in range(NJ):
                j = c
                nlive = 4 * (j + 1)
                # filler queues: QK of c+2 first (its copies gate the rope
                # filler), then V of c+1, then the previous chunk's o_proj;
                # rope(c+2) leads the hp1 queue.
                hp0_fill = deque()
                hp1_fill = deque()
                if c == 0:
                    hp0_fill.append(v_chain(2))
                    hp0_fill.append(v_chain(3))
                    if NJ > 1:
                        hp0_fill.append(rope(1))
                if c + 2 < NJ:
                    for dst, w_sb in ((qt, wq_sb), (kt, wk_sb)):
                        for m in range(2):
                            hp0_fill.append(qk_chain(c + 2, dst, w_sb, m))
                if c + 1 < NJ:
                    for t in range(4 * (c + 1), 4 * (c + 1) + 4):
                        hp0_fill.append(v_chain(t))
                if j > 0:
                    for t4 in range(2):
                        hp0_fill.append(oproj_unit(j - 1, t4))
                if c + 2 < NJ:
                    hp1_fill.append(rope(c + 2))
                if j > 0:
                    for t4 in range(2, 4):
                        hp1_fill.append(oproj_unit(j - 1, t4))

                for hp in range(2):
                    fill = hp0_fill if hp == 0 else hp1_fill
                    if hp == 1:
                        while hp0_fill:
                            hp1_fill.appendleft(hp0_fill.pop())
                    slots = nlive
                    otA = ps_ot.tile([128, 512], F32, tag="ot")
                    otB = ps_ot.tile([128, 512], F32, tag="ot")
                    jb = hp * SEQ + j * 512
                    ets = {}

                    def emit_st_exp(i, jb=jb, hp=hp, j=j, ets=ets):
                        r = i - 4 * j
                        c0 = 128 * r if r >= 0 else 0
                        ib = hp * SEQ + i * 128
                        st = ps_st.tile([128, 1024], F32, tag="st")
                        nc.tensor.matmul(st[:, c0:512],
                                         kt[0:64, ib:ib + 128],
                                         qt[0:64, jb + c0:jb + 512],
                                         start=True, stop=True)
                        nc.tensor.matmul(st[:, 512 + c0:1024],
                                         kt[64:128, ib:ib + 128],
                                         qt[64:128, jb + c0:jb + 512],
                                         start=True, stop=True)
                        et = etp.tile([128, 1024], DT_MM, tag="et")
                        if r >= 0:
                            stv = st[:].rearrange("p (g s) -> p g s", g=2)
                            etv = et[:].rearrange("p (g s) -> p g s", g=2)
                            nc.scalar.activation(etv[:, :, c0:512],
                                                 stv[:, :, c0:512],
                                                 AF.Exp, scale=0.125)
                            nc.vector.tensor_mul(
                                etv[:, :, c0:c0 + 128],
                                etv[:, :, c0:c0 + 128],
                                tri[:].rearrange("p (g d) -> p g d", g=2))
                        else:
                            nc.scalar.activation(et[:], st[:],
                                                 AF.Exp, scale=0.125)
                        ets[i] = (et, c0)
                        if DEBUG_DUMP and j == 1 and hp == 0:
                            nc.sync.dma_start(
                                out=dbg_et[:, i * 1024:(i + 1) * 1024],
                                in_=et[:])

                    def emit_pv(i, hp=hp, ets=ets, otA=otA, otB=otB,
                                nlive=nlive):
                        et, c0 = ets.pop(i)
                        vb = i * (HPC * 128) + 2 * hp * 128
                        nc.tensor.matmul(otA[:, c0:512],
                                         v_sb[:, vb:vb + 128],
                                         et[:, c0:512],
                                         start=(i == 0), stop=(i == nlive - 1))
                        nc.tensor.matmul(otB[:, c0:512],
                                         v_sb[:, vb + 128:vb + 256],
                                         et[:, 512 + c0:1024],
                                         start=(i == 0), stop=(i == nlive - 1))

                    n_fill0 = len(fill)
                    emitted = 0
                    for i in range(nlive):
                        emit_st_exp(i)
                        if i >= 1:
                            want = (n_fill0 * i + slots - 1) // slots
                            while fill and emitted < want:
                                fill.popleft()()
                                emitted += 1
                            emit_pv(i - 1)
                    while fill:
                        fill.popleft()()
                    emit_pv(nlive - 1)
                    if DEBUG_DUMP and j == 1 and hp == 0:
                        dbo = etp.tile([128, 1024], F32, tag="dbo",
                                       bufs=1)
                        nc.vector.tensor_copy(dbo[:, 0:512], otA[:])
                        nc.vector.tensor_copy(dbo[:, 512:1024], otB[:])
                        nc.sync.dma_start(out=dbg_ot[:], in_=dbo[:])
                    last = (j == NJ - 1 and hp == 1)
                    normalize(j, hp, otA, otB, split=4 if last else 1)

            for t4 in range(4):
                oproj_unit(NJ - 1, t4, fine=True)()

            if DEBUG_DUMP:
                nc.sync.dma_start(out=dbg_v[:], in_=v_sb[:])
                nc.sync.dma_start(out=dbg_ht[:], in_=ht[:])
                nc.sync.dma_start(out=dbg_qt[:], in_=qt[:])
                nc.sync.dma_start(out=dbg_kt[:], in_=kt[:])

    nc.compile()
    return nc


def _round_mm(a):
    return np.ascontiguousarray(a, dtype=np.float16)


def _prep_inputs(x, Wq, Wk, Wv, Wo, token_positions):
    x = np.asarray(x, dtype=np.float32)
    Wq = np.asarray(Wq, dtype=np.float32)
    Wk = np.asarray(Wk, dtype=np.float32)
    Wv = np.asarray(Wv, dtype=np.float32)
    Wo = np.asarray(Wo, dtype=np.float32)
    pos = np.asarray(token_positions).astype(np.float32)

    inv = 1.0 / (ROPE_THETA ** (np.arange(0, DK, 2, dtype=np.float32) / DK))
    freqs = pos[:, None] * inv[None, :]              # [SEQ, 32]
    cos_t, sin_t = np.cos(freqs).T, np.sin(freqs).T  # [32, SEQ]
    cosf = np.ascontiguousarray(np.tile(cos_t, (4, 1)), dtype=np.float16)
    sinf = np.tile(sin_t, (4, 1)).astype(np.float32)
    sinf[0:32] *= -1.0   # evens block gets -sin; odds +sin
    sinf[64:96] *= -1.0
    sinf = np.ascontiguousarray(sinf, dtype=np.float16)

    perm = np.concatenate([np.arange(0, 64, 2), np.arange(1, 64, 2)])
    in_maps = []
    for c in range(NCORES):
        b, g = divmod(c, GROUPS)
        rows = slice(g * DH, (g + 1) * DH)
        wq_s = Wq[rows, :].reshape(HPC, DK, DM)[:, perm, :].reshape(DH, DM)
        wk_s = Wk[rows, :].reshape(HPC, DK, DM)[:, perm, :].reshape(DH, DM)
        in_maps.append({
            "xt": _round_mm(x[b].T),
            "wq": _round_mm(wq_s.T),
            "wk": _round_mm(wk_s.T),
            "wv": _round_mm(Wv[rows, :].T),
            "wo": _round_mm(Wo[:, rows].T),
            "cosf": cosf,
            "sinf": sinf,
        })
    return in_maps


def kernel(x, Wq, Wk, Wv, Wo, token_positions):
    global _NC, LAST_RESULTS
    if _NC is None:
        _NC = _build()
    in_maps = _prep_inputs(x, Wq, Wk, Wv, Wo, token_positions)
    res = run_bass_kernel_spmd(_NC, in_maps, list(range(NCORES)), trace=TRACE)
    LAST_RESULTS = res
    y = np.empty((BATCH, SEQ, DM), dtype=np.float32)
    for b in range(BATCH):
        acc = res.results[4 * b]["y"].astype(np.float32).copy()
        for g in range(1, GROUPS):
            acc += res.results[4 * b + g]["y"]
        y[b] = acc
    return y
